# revision 1
# baseline (speedup 1.0000x reference)
"""Distributed GAT (2-layer, PyG GATConv semantics) as a Bass/Tile SPMD kernel
for 8 Trainium2 NeuronCores.

Sharding: nodes row-sharded across cores; edges sharded by dst. Per layer each
core computes table_local = x_local @ [W | W@As | W@Ad | 0pad] ([npc, 320]:
h | alpha_src | alpha_dst | pad), AllGather -> full table [N, 320], plus a
compact local alpha_dst table adt [npc, 64]. Edge phase per group of 128 dst
nodes (edges dst-sorted, split into src<32768 (A) / src>=32768 (B) sections
for int16 dma_gather indices, each padded to a core-uniform subtile count):
  dma_gather [h|asrc] rows by src (1280B elems),
  dma_gather adst rows by local dst (256B elems),
  e = exp(leakyrelu(asrc+adst))  (softmax without max-subtraction: alphas are
      O(5) so exp is safe in f32; mathematically identical),
  w = h * e (head-wise),
  segment-sum via PE matmuls with on-device-built selector S (is_equal vs
      iota) into a PSUM group accumulator [128, 264] = [sum w | sum e],
  flush: x' = relu(sum_w / sum_e + bias) -> feeds next layer table (or head).
Final head: logits = x3 @ Wc + bc, log_softmax, row-sharded output.
"""
import math
import numpy as np

import concourse.bass as bass
import concourse.bacc as bacc
import concourse.tile as tile
from concourse import mybir
from concourse.masks import make_identity

F32 = mybir.dt.float32
F32R = mybir.dt.float32r
I16 = mybir.dt.int16

P = 128
SPLIT_AT = 32768          # int16 index limit for dma_gather


class Cfg:
    def __init__(self, N, DIN, H, C, OUT, n_cores):
        self.N, self.DIN, self.H, self.C, self.OUT = N, DIN, H, C, OUT
        self.HID = H * C
        self.GC = self.HID + H              # useful gathered cols: h | asrc
        self.RT = 320                       # padded table row (1280B % 256 == 0)
        self.AC = 64                        # adt row (256B)
        assert self.HID + 2 * H <= self.RT
        self.n_cores = n_cores
        assert N % n_cores == 0
        self.npc = N // n_cores
        assert self.npc <= SPLIT_AT, "local dst must fit int16"
        self.NB = math.ceil(self.npc / P)
        self.npc_pad = self.NB * P
        self.split = N > SPLIT_AT
        # filled by preprocess:
        self.UA = None
        self.UB = None

    @property
    def UT(self):
        return self.UA + self.UB


def _wrap_idx(vals):
    """int16 index list (len % 16 == 0) -> dma_gather wrapped layout
    [128, len/16]: index j at partition j%16 col j//16, replicated x8."""
    n = len(vals)
    w = vals.reshape(n // 16, 16).T.astype(np.int16)   # [16, n/16]
    return np.tile(w, (8, 1))                          # [128, n/16]


def preprocess(cfg: Cfg, edge_index: np.ndarray):
    """Per-core edge-stream arrays for the group-wise dma_gathers.

    Returns list per core of dict:
      idxA [NB, 128, UA*8] i16   (src < SPLIT_AT)
      idxB [NB, 128, UB*8] i16   (src - SPLIT_AT)   (only if cfg.split)
      idxD [NB, 128, UT*8] i16   (local dst, for adt)
      dstc [NB, 128, UT]   f32   (dst rel to group, -1 for padding)
    """
    N, n_cores, npc = cfg.N, cfg.n_cores, cfg.npc
    NB, npc_pad = cfg.NB, cfg.npc_pad

    src = np.concatenate([edge_index[0], np.arange(N, dtype=edge_index.dtype)])
    dst = np.concatenate([edge_index[1], np.arange(N, dtype=edge_index.dtype)])
    order = np.argsort(dst, kind="stable")
    src_s = np.asarray(src[order], dtype=np.int64)
    dst_s = np.asarray(dst[order], dtype=np.int64)
    bounds = np.searchsorted(dst_s, np.arange(n_cores + 1) * npc)

    cores = []
    UA = UB = 0
    for c in range(n_cores):
        lo, hi = bounds[c], bounds[c + 1]
        s_c = src_s[lo:hi]
        d_c = dst_s[lo:hi] - c * npc
        if npc_pad > npc:  # fake dst rows so every psum row has a real denom
            fake = np.arange(npc, npc_pad, dtype=np.int64)
            s_c = np.concatenate([s_c, np.zeros(len(fake), np.int64)])
            d_c = np.concatenate([d_c, fake])
        isB = (s_c >= SPLIT_AT) if cfg.split else np.zeros(len(s_c), bool)
        g_c = d_c // P
        # sort by (group, section, dst)
        key = (g_c * 2 + isB) * npc_pad + d_c
        o = np.argsort(key, kind="stable")
        s_c, d_c, g_c, isB = s_c[o], d_c[o], g_c[o], isB[o]
        cntA = np.bincount(g_c[~isB], minlength=NB)
        cntB = np.bincount(g_c[isB], minlength=NB)
        UA = max(UA, int(math.ceil(cntA.max() / P)))
        if cfg.split:
            UB = max(UB, int(math.ceil(cntB.max() / P)))
        cores.append((s_c, d_c, g_c, isB, cntA, cntB))
    cfg.UA, cfg.UB = UA, UB
    UT = UA + UB

    out = []
    for (s_c, d_c, g_c, isB, cntA, cntB) in cores:
        # target slot within the group stream [A pad to UA*128 | B pad to UB*128]
        startA = np.zeros(NB + 1, np.int64)
        np.cumsum(cntA, out=startA[1:])
        startB = np.zeros(NB + 1, np.int64)
        np.cumsum(cntB, out=startB[1:])
        rank = np.empty(len(d_c), np.int64)
        idxall = np.arange(len(d_c), dtype=np.int64)
        # positions: edges are sorted (group, section); rank within section:
        secA = ~isB
        rank[secA] = idxall[secA] - (startA[g_c[secA]] + startB[g_c[secA]])
        rank[isB] = idxall[isB] - (startA[g_c[isB] + 1] + startB[g_c[isB]])
        tgt = g_c * (UT * P) + np.where(isB, UA * P + rank, rank)

        srcv = np.zeros(NB * UT * P, np.int64)          # pad -> row 0
        dstl = np.zeros(NB * UT * P, np.int64)          # pad -> row 0
        dstcv = np.full(NB * UT * P, -1.0, np.float32)  # pad -> no dst
        srcv[tgt] = np.where(isB, s_c - SPLIT_AT, s_c)
        dstl[tgt] = np.where(d_c < npc, d_c, 0)         # fake dsts gather row 0
        dstcv[tgt] = (d_c % P).astype(np.float32)

        srcv = srcv.reshape(NB, UT * P)
        dstl = dstl.reshape(NB, UT * P)
        dstcv = dstcv.reshape(NB, UT, P)

        idxA = np.stack([_wrap_idx(srcv[g, :UA * P]) for g in range(NB)])
        idxD = np.stack([_wrap_idx(dstl[g]) for g in range(NB)])
        d = {
            "idxA": idxA.astype(np.int16),
            "idxD": idxD.astype(np.int16),
            # dstc[g, p, k] = value at stream pos k*128+p
            "dstc": np.ascontiguousarray(dstcv.transpose(0, 2, 1)),
        }
        if cfg.split:
            idxB = np.stack([_wrap_idx(srcv[g, UA * P:]) for g in range(NB)])
            d["idxB"] = idxB.astype(np.int16)
        out.append(d)
    return out


def expand_att(a, HID, H, C):
    A = np.zeros((HID, H), np.float32)
    for h in range(H):
        A[h * C:(h + 1) * C, h] = a[h]
    return A


def build_program(cfg: Cfg, edge_reps=1, no_collectives=False, pert=frozenset(),
                  gw_bufs=2, idx_bufs=2, sq=1, f32r=False, ch=8):
    """Emit the (core-uniform) SPMD program. Returns nc."""
    NB, UA, UB, UT = cfg.NB, cfg.UA, cfg.UB, cfg.UT
    RT, GC, AC = cfg.RT, cfg.GC, cfg.AC
    HID, OUT, DIN, H = cfg.HID, cfg.OUT, cfg.DIN, cfg.H
    npc, N = cfg.npc, cfg.N
    DC = DIN // P
    HC = HID // P
    NA_ROWS = min(N, SPLIT_AT)

    nc = bacc.Bacc("TRN2", target_bir_lowering=False, debug=False,
                   num_devices=cfg.n_cores, num_swdge_queues=sq)

    t_xT = nc.dram_tensor("xT", [DIN, npc], F32, kind="ExternalInput")
    t_M1 = nc.dram_tensor("M1", [DIN, RT], F32, kind="ExternalInput")
    t_M2 = nc.dram_tensor("M2", [HID, RT], F32, kind="ExternalInput")
    t_Wc = nc.dram_tensor("Wc", [HID, OUT], F32, kind="ExternalInput")
    t_b1 = nc.dram_tensor("b1", [P, HID], F32, kind="ExternalInput")
    t_b2 = nc.dram_tensor("b2", [P, HID], F32, kind="ExternalInput")
    t_bc = nc.dram_tensor("bc", [P, OUT], F32, kind="ExternalInput")
    t_iota = nc.dram_tensor("iota", [P, P], F32, kind="ExternalInput")
    t_idxA = nc.dram_tensor("idxA", [NB, P, UA * 8], I16, kind="ExternalInput")
    if cfg.split:
        t_idxB = nc.dram_tensor("idxB", [NB, P, UB * 8], I16,
                                kind="ExternalInput")
    t_idxD = nc.dram_tensor("idxD", [NB, P, UT * 8], I16, kind="ExternalInput")
    t_dstc = nc.dram_tensor("dstc", [NB, P, UT], F32, kind="ExternalInput")
    t_out = nc.dram_tensor("out", [npc, OUT], F32, kind="ExternalOutput")

    rgroups = [list(range(cfg.n_cores))]

    with tile.TileContext(nc) as tc:
        with (
            tc.tile_pool(name="const", bufs=1) as cp,
            tc.tile_pool(name="xt", bufs=2) as xtp,
            tc.tile_pool(name="tbl", bufs=3) as tblp,
            tc.tile_pool(name="gw", bufs=gw_bufs) as gwp,
            tc.tile_pool(name="idx", bufs=idx_bufs) as idxp,
            tc.tile_pool(name="ad", bufs=gw_bufs) as adp,
            tc.tile_pool(name="zz", bufs=3) as zzp,
            tc.tile_pool(name="smat", bufs=gw_bufs) as sp,
            tc.tile_pool(name="xb", bufs=2) as xbp,
            tc.tile_pool(name="xtb", bufs=2) as xtbp,
            tc.tile_pool(name="hd", bufs=2) as hdp,
            tc.tile_pool(name="ps_acc", bufs=2, space="PSUM") as ps_acc,
            tc.tile_pool(name="ps_tp", bufs=2, space="PSUM") as ps_tp,
            tc.tile_pool(name="ps_tb", bufs=2, space="PSUM") as ps_tb,
            tc.tile_pool(name="ps_lg", bufs=2, space="PSUM") as ps_lg,
            tc.tile_pool(name="dram", bufs=1, space="DRAM") as dp,
        ):
            # ---- constants ----
            M1sb = cp.tile([P, DC, RT], F32)
            M2sb = cp.tile([P, HC, RT], F32)
            WcSb = cp.tile([P, HC, OUT], F32)
            b1sb = cp.tile([P, HID], F32)
            b2sb = cp.tile([P, HID], F32)
            bcsb = cp.tile([P, OUT], F32)
            iota = cp.tile([P, P], F32)
            ident = cp.tile([P, P], F32)
            nc.sync.dma_start(out=M1sb[:], in_=t_M1[:, :].rearrange(
                "(a c) r -> c a r", c=P))
            nc.sync.dma_start(out=M2sb[:], in_=t_M2[:, :].rearrange(
                "(a c) r -> c a r", c=P))
            nc.sync.dma_start(out=WcSb[:], in_=t_Wc[:, :].rearrange(
                "(a c) r -> c a r", c=P))
            nc.sync.dma_start(out=b1sb[:], in_=t_b1[:, :])
            nc.sync.dma_start(out=b2sb[:], in_=t_b2[:, :])
            nc.sync.dma_start(out=bcsb[:], in_=t_bc[:, :])
            nc.sync.dma_start(out=iota[:], in_=t_iota[:, :])
            make_identity(nc, ident[:])

            # ---- internal DRAM ----
            ag1_in = dp.tile([npc, RT], F32)
            ag2_in = dp.tile([npc, RT], F32)
            table1 = dp.tile([N, RT], F32)
            table2 = dp.tile([N, RT], F32)
            adt1 = dp.tile([npc, AC], F32)
            adt2 = dp.tile([npc, AC], F32)

            # ---- phase B: layer-1 table ----
            for b in range(NB):
                ncols = min(P, npc - b * P)
                xt = xtp.tile([P, DC, P], F32, name="xt")
                nc.sync.dma_start(
                    out=xt[:, :, 0:ncols],
                    in_=t_xT[:, b * P:b * P + ncols].rearrange(
                        "(a c) n -> c a n", c=P))
                pstb = ps_tb.tile([P, RT], F32, name="pstb", tag="pstb")
                for a in range(DC):
                    lh, rh = xt[:, a, 0:ncols], M1sb[:, a, :]
                    if f32r:
                        lh, rh = lh.bitcast(F32R), rh.bitcast(F32R)
                    nc.tensor.matmul(pstb[0:ncols, :], lh, rh, start=(a == 0),
                                     stop=(a == DC - 1))
                tbs = tblp.tile([P, RT], F32, name="tbs")
                nc.vector.tensor_copy(out=tbs[0:ncols, :], in_=pstb[0:ncols, :])
                nc.sync.dma_start(out=ag1_in[b * P:b * P + ncols, :],
                                  in_=tbs[0:ncols, :])
                nc.sync.dma_start(out=adt1[b * P:b * P + ncols, :],
                                  in_=tbs[0:ncols, HID:HID + AC])

            if no_collectives:
                nc.sync.dma_start(out=table1[0:npc, :], in_=ag1_in[:])
            else:
                nc.gpsimd.collective_compute(
                    "AllGather", mybir.AluOpType.bypass, replica_groups=rgroups,
                    ins=[ag1_in[:].opt()], outs=[table1[:].opt()])

            # ---- edge phase (shared by both layers) ----
            def edge_phase(table_full, adt, t_idxB, flush_fn):
                qn = [0]
                for g in range(NB):
                    ia = idxp.tile([P, UA * 8], I16, name="ia")
                    nc.sync.dma_start(out=ia[:], in_=t_idxA[g])
                    idt = idxp.tile([P, UT * 8], I16, name="idt")
                    nc.sync.dma_start(out=idt[:], in_=t_idxD[g])
                    dc = idxp.tile([P, UT], F32, name="dc")
                    nc.sync.dma_start(out=dc[:], in_=t_dstc[g])
                    CH = ch  # subtiles per dma_gather (ucode ring: <=1024 idxs)

                    def chunked_gather(dst_tile, src_ap, idx_tile, u, elem):
                        for c0 in range(0, u, CH):
                            c1 = min(c0 + CH, u)
                            n = (c1 - c0) * P
                            qn[0] = (qn[0] + 1) % sq
                            if "densegather" in pert:
                                nc.sync.dma_start(
                                    out=dst_tile[:, c0:c1, :],
                                    in_=src_ap.tensor.ap()[0:n, 0:elem].rearrange(
                                        "(k p) r -> p k r", p=P))
                            else:
                                nc.gpsimd.dma_gather(
                                    dst_tile[:, c0:c1, :], src_ap,
                                    idx_tile[:, c0 * 8:c1 * 8], n, n, elem,
                                    queue_num=qn[0])

                    gA = gwp.tile([P, UA, RT], F32, name="gA")
                    chunked_gather(gA, table_full[0:NA_ROWS, :], ia, UA, RT)
                    tiles = [(gA, UA)]
                    if cfg.split:
                        ib = idxp.tile([P, UB * 8], I16, name="ib")
                        nc.sync.dma_start(out=ib[:], in_=t_idxB[g])
                        gB = gwp.tile([P, UB, RT], F32, name="gB")
                        chunked_gather(gB, table_full[SPLIT_AT:N, :], ib, UB, RT)
                        tiles.append((gB, UB))
                    ad = adp.tile([P, UT, AC], F32, name="ad")
                    chunked_gather(ad, adt[:], idt, UT, AC)
                    # e = exp(lrelu(asrc + adst)); w = h * e
                    zt = zzp.tile([P, UT, H], F32, name="zt")
                    off = 0
                    for (gt, u) in tiles:
                        nc.vector.tensor_add(
                            out=zt[:, off:off + u, :],
                            in0=gt[:, :, HID:GC],
                            in1=ad[:, off:off + u, H:2 * H])
                        off += u
                    nc.vector.scalar_tensor_tensor(
                        out=zt[:], in0=zt[:], scalar=0.2, in1=zt[:],
                        op0=mybir.AluOpType.mult, op1=mybir.AluOpType.max)
                    off = 0
                    for (gt, u) in tiles:
                        nc.scalar.activation(gt[:, :, HID:GC],
                                             zt[:, off:off + u, :],
                                             mybir.ActivationFunctionType.Exp)
                        if "nowmul" not in pert:
                            e_b = gt[:, :, HID:GC].to_broadcast([P, u, H, cfg.C])
                            hv = gt[:, :, 0:HID].rearrange(
                                "p k (h c) -> p k h c", c=cfg.C)
                            nc.vector.tensor_mul(out=hv, in0=hv, in1=e_b)
                        off += u
                    St = sp.tile([P, UT, P], F32, name="St")
                    if "nosbuild" not in pert:
                        nc.vector.tensor_tensor(
                            out=St[:], in0=dc[:].to_broadcast([P, UT, P]),
                            in1=iota[:].rearrange("p (o i) -> p o i", o=1
                                                  ).to_broadcast([P, UT, P]),
                            op=mybir.AluOpType.is_equal)
                    acc = ps_acc.tile([P, GC], F32, name="acc")
                    for j in ([0] if "mm1" in pert else range(UT)):
                        gt, k = (gA, j) if j < UA else (gB, j - UA)
                        lh = St[:, j, :]
                        rh = gt[:, k, 0:GC]
                        if f32r:
                            lh, rh = lh.bitcast(F32R), rh.bitcast(F32R)
                        nc.tensor.matmul(acc[:], lh, rh,
                                         start=(j == 0),
                                         stop=(j == UT - 1) or "mm1" in pert)
                    if "noflush" not in pert:
                        flush_fn(g, acc)

            # ---- flush helpers ----
            def normalize(acc, bias_sb):
                rec = zzp.tile([P, H], F32, name="rec")
                nc.vector.reciprocal(rec[:], acc[:, HID:GC])
                xb = xbp.tile([P, HID], F32, name="xb")
                nc.vector.tensor_mul(
                    out=xb[:].rearrange("p (h c) -> p h c", c=cfg.C),
                    in0=acc[:, 0:HID].rearrange("p (h c) -> p h c", c=cfg.C),
                    in1=rec[:].to_broadcast([P, H, cfg.C]))
                nc.vector.tensor_add(out=xb[:], in0=xb[:], in1=bias_sb[:])
                nc.vector.tensor_scalar_max(xb[:], xb[:], 0.0)
                return xb

            def transpose2(xb):
                outs = []
                for a in range(HC):
                    pst = ps_tp.tile([P, P], F32, name="pst")
                    nc.tensor.transpose(pst[:], xb[:, a * P:(a + 1) * P],
                                        ident[:])
                    xts = xtbp.tile([P, P], F32, name="xts")
                    nc.vector.tensor_copy(out=xts[:], in_=pst[:])
                    outs.append(xts)
                return outs

            def flush_layer1(g, acc):
                ng = min(P, npc - g * P)
                xb = normalize(acc, b1sb)
                xts = transpose2(xb)
                pstb = ps_tb.tile([P, RT], F32, name="pstb2", tag="pstb")
                for a in range(HC):
                    lh, rh = xts[a][:, 0:ng], M2sb[:, a, :]
                    if f32r:
                        lh, rh = lh.bitcast(F32R), rh.bitcast(F32R)
                    nc.tensor.matmul(pstb[0:ng, :], lh, rh, start=(a == 0),
                                     stop=(a == HC - 1))
                tbs = tblp.tile([P, RT], F32, name="tbs2")
                nc.vector.tensor_copy(out=tbs[0:ng, :], in_=pstb[0:ng, :])
                nc.sync.dma_start(out=ag2_in[g * P:g * P + ng, :],
                                  in_=tbs[0:ng, :])
                nc.sync.dma_start(out=adt2[g * P:g * P + ng, :],
                                  in_=tbs[0:ng, HID:HID + AC])

            def flush_layer2(g, acc):
                ng = min(P, npc - g * P)
                xb = normalize(acc, b2sb)
                xts = transpose2(xb)
                pslg = ps_lg.tile([P, OUT], F32, name="pslg")
                for a in range(HC):
                    nc.tensor.matmul(pslg[0:ng, :], xts[a][:, 0:ng],
                                     WcSb[:, a, :], start=(a == 0),
                                     stop=(a == HC - 1))
                lg = hdp.tile([P, OUT], F32, name="lg")
                nc.vector.tensor_add(out=lg[0:ng, :], in0=pslg[0:ng, :],
                                     in1=bcsb[0:ng, :])
                mx = hdp.tile([P, 1], F32, name="mx")
                nc.vector.tensor_reduce(out=mx[0:ng, :], in_=lg[0:ng, :],
                                        axis=mybir.AxisListType.X,
                                        op=mybir.AluOpType.max)
                nc.vector.tensor_sub(out=lg[0:ng, :], in0=lg[0:ng, :],
                                     in1=mx[0:ng, :].to_broadcast([ng, OUT]))
                ex = hdp.tile([P, OUT], F32, name="ex")
                dn = hdp.tile([P, 1], F32, name="dn")
                nc.scalar.activation(ex[0:ng, :], lg[0:ng, :],
                                     mybir.ActivationFunctionType.Exp,
                                     accum_out=dn[0:ng, :])
                lnd = hdp.tile([P, 1], F32, name="lnd")
                nc.scalar.activation(lnd[0:ng, :], dn[0:ng, :],
                                     mybir.ActivationFunctionType.Ln)
                ob = hdp.tile([P, OUT], F32, name="ob")
                nc.vector.tensor_sub(out=ob[0:ng, :], in0=lg[0:ng, :],
                                     in1=lnd[0:ng, :].to_broadcast([ng, OUT]))
                nc.sync.dma_start(out=t_out[g * P:g * P + ng, :],
                                  in_=ob[0:ng, :])

            for _ in range(edge_reps):
                edge_phase(table1, adt1, t_idxB if cfg.split else None,
                           flush_layer1)
            if no_collectives:
                nc.sync.dma_start(out=table2[0:npc, :], in_=ag2_in[:])
            else:
                nc.gpsimd.collective_compute(
                    "AllGather", mybir.AluOpType.bypass, replica_groups=rgroups,
                    ins=[ag2_in[:].opt()], outs=[table2[:].opt()])
            for _ in range(edge_reps):
                edge_phase(table2, adt2, t_idxB if cfg.split else None,
                           flush_layer2)

    nc.compile()
    return nc


def make_in_maps(cfg: Cfg, pre, x, W1, as1, ad1, b1, W2, as2, ad2, b2, Wc, bc):
    H, C, HID, npc, RT = cfg.H, cfg.C, cfg.HID, cfg.npc, cfg.RT

    def mk_m(W, a_s, a_d):
        M = np.zeros((W.shape[0], RT), np.float32)
        M[:, 0:HID] = W
        M[:, HID:HID + H] = W @ expand_att(a_s, HID, H, C)
        M[:, HID + H:HID + 2 * H] = W @ expand_att(a_d, HID, H, C)
        return M

    M1 = mk_m(W1, as1, ad1)
    M2 = mk_m(W2, as2, ad2)
    iota = np.tile(np.arange(P, dtype=np.float32)[None, :], (P, 1))
    maps = []
    for c in range(cfg.n_cores):
        m = {
            "xT": np.ascontiguousarray(x[c * npc:(c + 1) * npc].T,
                                       dtype=np.float32),
            "M1": M1, "M2": M2, "Wc": Wc.astype(np.float32),
            "b1": np.tile(b1[None, :], (P, 1)).astype(np.float32),
            "b2": np.tile(b2[None, :], (P, 1)).astype(np.float32),
            "bc": np.tile(bc[None, :], (P, 1)).astype(np.float32),
            "iota": iota,
            "idxA": pre[c]["idxA"], "idxD": pre[c]["idxD"],
            "dstc": pre[c]["dstc"],
        }
        if cfg.split:
            m["idxB"] = pre[c]["idxB"]
        maps.append(m)
    return maps


# ---------------------------------------------------------------------------
# Harness entry point: full inputs in, full output out.
# ---------------------------------------------------------------------------

def kernel(x, edge_index, W1, as1, ad1, b1, W2, as2, ad2, b2, Wc, bc):
    x = np.asarray(x, dtype=np.float32)
    edge_index = np.asarray(edge_index)
    N, DIN = x.shape
    H, C = np.asarray(as1).shape
    OUT = np.asarray(Wc).shape[1]
    n_cores = 8

    cfg = Cfg(N, DIN, H, C, OUT, n_cores)
    pre = preprocess(cfg, edge_index)
    nc = build_program(cfg, gw_bufs=3, idx_bufs=4, sq=4)
    in_maps = make_in_maps(cfg, pre, x,
                           np.asarray(W1, np.float32), np.asarray(as1, np.float32),
                           np.asarray(ad1, np.float32), np.asarray(b1, np.float32),
                           np.asarray(W2, np.float32), np.asarray(as2, np.float32),
                           np.asarray(ad2, np.float32), np.asarray(b2, np.float32),
                           np.asarray(Wc, np.float32), np.asarray(bc, np.float32))

    from concourse import bass_utils
    last_err = None
    for _attempt in range(3):   # a wedged device from a prior crash can fail once
        try:
            res = bass_utils.run_bass_kernel_spmd(nc, in_maps,
                                                  core_ids=list(range(n_cores)))
            break
        except Exception as e:                      # noqa: BLE001
            last_err = e
    else:
        raise last_err
    return np.concatenate([res.results[c]["out"] for c in range(n_cores)],
                          axis=0)



# revision 3
# speedup vs baseline: 1.6010x; 1.6010x over previous
"""Distributed GAT (2-layer, PyG GATConv semantics) as a Bass/Tile SPMD
kernel for 8 Trainium2 NeuronCores. v2: bf16 gather table, no adt gather,
bf16 PE path, host-shipped transposed selectors, Shared-space tables.

Sharding: nodes row-sharded across 8 cores; edges sharded by dst. Per layer
each core computes table_local = [h bf16 (256) | asrc f32 (8) | pad] rows of
384 bf16 (768B, dma_gather-able), AllGather -> full table [N, 384] bf16, plus
a per-core SBUF-resident adt tile [P, NB*H] f32 of alpha_dst values.

Edge phase per group of 128 dst nodes (edges dst-sorted, A/B split by
src<32768 for int16 gather indices, padded to core-uniform subtile counts):
  dma_gather 768B rows by src,
  St  [e-part, d-col] = is_equal(dstc, iota)        (segment-sum selector)
  StT [d-part, e-col] = (iota>=st)&(iota<en)        (host-shipped run ranges;
      edges are dst-sorted so each (d, subtile) is one contiguous run)
  adstE[e, h] = StT_j^T @ adst_g                    (PE broadcast, 8-col mm)
  e = exp(lrelu(asrc + adstE)); zw = [h*e | e] bf16,
  acc[d, 264] += St_j^T @ zw_j                      (bf16 segment matmul)
  flush: x' = relu(sum_w / sum_e + bias) -> next-layer table row (bf16
      transposes + bf16 matmul vs M2) or the log_softmax head.
"""
import math
import numpy as np

import concourse.bass as bass
import concourse.bacc as bacc
import concourse.tile as tile
from concourse import mybir
from concourse.masks import make_identity

F32 = mybir.dt.float32
BF16 = mybir.dt.bfloat16
I16 = mybir.dt.int16

P = 128
SPLIT_AT = 32768


class Cfg:
    def __init__(self, N, DIN, H, C, OUT, n_cores):
        self.N, self.DIN, self.H, self.C, self.OUT = N, DIN, H, C, OUT
        self.HID = H * C
        self.RC = self.HID + 2 * H          # psum compute row: h|asrc|adst
        self.RT = 384                       # bf16 table row (768B)
        assert (self.HID + 2 * H) * 2 <= self.RT * 2
        self.n_cores = n_cores
        assert N % n_cores == 0
        self.npc = N // n_cores
        assert self.npc <= SPLIT_AT
        self.NB = math.ceil(self.npc / P)
        self.npc_pad = self.NB * P
        self.split = N > SPLIT_AT
        self.UA = None
        self.UB = None

    @property
    def UT(self):
        return self.UA + self.UB


def _wrap_idx(vals):
    n = len(vals)
    w = vals.reshape(n // 16, 16).T.astype(np.int16)
    return np.tile(w, (8, 1))


def preprocess(cfg: Cfg, edge_index: np.ndarray):
    """Per-core edge-stream arrays.

    Returns list per core of dict:
      idx  [NB, 128, (UA+UB)*8] i16  (A: src, B: src - SPLIT_AT)
      dse  [NB, 128, 3*UT] bf16      (dstc | run-start | run-end)
    """
    N, n_cores, npc = cfg.N, cfg.n_cores, cfg.npc
    NB, npc_pad = cfg.NB, cfg.npc_pad

    src = np.concatenate([edge_index[0], np.arange(N, dtype=edge_index.dtype)])
    dst = np.concatenate([edge_index[1], np.arange(N, dtype=edge_index.dtype)])
    order = np.argsort(dst, kind="stable")
    src_s = np.asarray(src[order], dtype=np.int64)
    dst_s = np.asarray(dst[order], dtype=np.int64)
    bounds = np.searchsorted(dst_s, np.arange(n_cores + 1) * npc)

    cores = []
    UA = UB = 0
    for c in range(n_cores):
        lo, hi = bounds[c], bounds[c + 1]
        s_c = src_s[lo:hi]
        d_c = dst_s[lo:hi] - c * npc
        if npc_pad > npc:
            fake = np.arange(npc, npc_pad, dtype=np.int64)
            s_c = np.concatenate([s_c, np.zeros(len(fake), np.int64)])
            d_c = np.concatenate([d_c, fake])
        isB = (s_c >= SPLIT_AT) if cfg.split else np.zeros(len(s_c), bool)
        g_c = d_c // P
        key = (g_c * 2 + isB) * npc_pad + d_c
        o = np.argsort(key, kind="stable")
        s_c, d_c, g_c, isB = s_c[o], d_c[o], g_c[o], isB[o]
        cntA = np.bincount(g_c[~isB], minlength=NB)
        cntB = np.bincount(g_c[isB], minlength=NB)
        UA = max(UA, int(math.ceil(cntA.max() / P)))
        if cfg.split:
            UB = max(UB, int(math.ceil(cntB.max() / P)))
        cores.append((s_c, d_c, g_c, isB, cntA, cntB))
    cfg.UA, cfg.UB = UA, UB
    UT = UA + UB

    out = []
    for (s_c, d_c, g_c, isB, cntA, cntB) in cores:
        startA = np.zeros(NB + 1, np.int64)
        np.cumsum(cntA, out=startA[1:])
        startB = np.zeros(NB + 1, np.int64)
        np.cumsum(cntB, out=startB[1:])
        rank = np.empty(len(d_c), np.int64)
        idxall = np.arange(len(d_c), dtype=np.int64)
        secA = ~isB
        rank[secA] = idxall[secA] - (startA[g_c[secA]] + startB[g_c[secA]])
        rank[isB] = idxall[isB] - (startA[g_c[isB] + 1] + startB[g_c[isB]])
        tgt = g_c * (UT * P) + np.where(isB, UA * P + rank, rank)

        srcv = np.zeros(NB * UT * P, np.int64)
        dstcv = np.full(NB * UT * P, -1.0, np.float32)
        srcv[tgt] = np.where(isB, s_c - SPLIT_AT, s_c)
        dstcv[tgt] = (d_c % P).astype(np.float32)

        # run ranges: for (g, d, j) the slots of dst d in subtile j are
        # contiguous (dst-sorted within each section; subtiles don't span
        # sections). empty -> st=128, en=0 -> all-zero StT row.
        v = dstcv.reshape(NB, UT, P)
        gg, jj, ee = np.meshgrid(np.arange(NB), np.arange(UT), np.arange(P),
                                 indexing="ij")
        valid = v >= 0
        vi = v[valid].astype(np.int64)
        st3 = np.full((NB, P, UT), 128, np.int64)
        en3 = np.full((NB, P, UT), -1, np.int64)
        np.minimum.at(st3, (gg[valid], vi, jj[valid]), ee[valid])
        np.maximum.at(en3, (gg[valid], vi, jj[valid]), ee[valid])
        en3 = en3 + 1

        srcv = srcv.reshape(NB, UT * P)
        idx = np.stack([_wrap_idx(srcv[g]) for g in range(NB)])
        dse = np.concatenate([
            np.ascontiguousarray(v.transpose(0, 2, 1)),   # dstc [NB, P, UT]
            st3.astype(np.float32),
            en3.astype(np.float32),
        ], axis=2)
        # selector bitmaps: StT [d-part, (j, e)] and St [e-part, (j, d)]
        ee128 = np.arange(P)
        stt = ((ee128[None, None, None, :] >= st3[:, :, :, None])
               & (ee128[None, None, None, :] < en3[:, :, :, None]))
        stm = (v[:, :, None, :] == ee128[None, None, :, None])  # [g, j, d, e]
        out.append({
            "idx": idx.astype(np.int16),
            "dse": dse,   # f32 here; cast to bf16 in make_in_maps
            "stt": stt.astype(np.float32),                   # [NB, P, UT, P]
            "stm": np.ascontiguousarray(
                stm.transpose(0, 3, 1, 2)).astype(np.float32),  # [NB,P,UT,P]
        })
    return out


def expand_att(a, HID, H, C):
    A = np.zeros((HID, H), np.float32)
    for h in range(H):
        A[h * C:(h + 1) * C, h] = a[h]
    return A


def build_program(cfg: Cfg, edge_reps=1, no_collectives=False, pert=frozenset(),
                  gw_bufs=3, idx_bufs=4, sq=4, ch=8, zw_bufs=2, sp_bufs=2,
                  dual_acc=0, sel_ship=0, skew=0, act_lrelu=0, stt_q=0,
                  dve_cyc=0.0, kernel_reps=1, shared_tables=0):
    args = (cfg, edge_reps, no_collectives, pert, gw_bufs, idx_bufs, sq, ch,
            zw_bufs, sp_bufs, dual_acc, sel_ship, skew, act_lrelu, stt_q,
            kernel_reps, shared_tables)
    if dve_cyc:
        from concourse import hw_specs
        old_cyc = dict(hw_specs.TRN2Spec.CYCLE_T)
        hw_specs.TRN2Spec.CYCLE_T = {**old_cyc,
                                     mybir.EngineType.DVE: dve_cyc}
        try:
            return _build_program(*args)
        finally:
            hw_specs.TRN2Spec.CYCLE_T = old_cyc
    return _build_program(*args)


def _build_program(cfg, edge_reps, no_collectives, pert, gw_bufs, idx_bufs,
                   sq, ch, zw_bufs, sp_bufs, dual_acc, sel_ship, skew,
                   act_lrelu, stt_q, kernel_reps=1, shared_tables=0):
    NB, UA, UB, UT = cfg.NB, cfg.UA, cfg.UB, cfg.UT
    RT, RC = cfg.RT, cfg.RC
    HID, OUT, DIN, H, C = cfg.HID, cfg.OUT, cfg.DIN, cfg.H, cfg.C
    npc, N = cfg.npc, cfg.N
    DC = DIN // P
    HC = HID // P
    NA_ROWS = min(N, SPLIT_AT)

    nc = bacc.Bacc("TRN2", target_bir_lowering=False, debug=False,
                   num_devices=cfg.n_cores, num_swdge_queues=sq)

    t_xT = nc.dram_tensor("xT", [DIN, npc], BF16, kind="ExternalInput")
    t_M1 = nc.dram_tensor("M1", [DIN, RC], BF16, kind="ExternalInput")
    t_M2 = nc.dram_tensor("M2", [HID, RC], BF16, kind="ExternalInput")
    t_Wc = nc.dram_tensor("Wc", [HID, OUT], BF16, kind="ExternalInput")
    t_b1 = nc.dram_tensor("b1", [P, HID], F32, kind="ExternalInput")
    t_b2 = nc.dram_tensor("b2", [P, HID], F32, kind="ExternalInput")
    t_bc = nc.dram_tensor("bc", [P, OUT], F32, kind="ExternalInput")
    t_iota = nc.dram_tensor("iota", [P, P], BF16, kind="ExternalInput")
    t_idx = nc.dram_tensor("idx", [NB, P, UT * 8], I16, kind="ExternalInput")
    t_dse = nc.dram_tensor("dse", [NB, P, 3 * UT], BF16, kind="ExternalInput")
    if sel_ship >= 1:
        t_stt = nc.dram_tensor("stt", [NB, P, UT * P], BF16,
                               kind="ExternalInput")
    if sel_ship >= 2:
        t_stm = nc.dram_tensor("stm", [NB, P, UT * P], BF16,
                               kind="ExternalInput")
    t_out = nc.dram_tensor("out", [npc, OUT], F32, kind="ExternalOutput")

    rgroups = [list(range(cfg.n_cores))]

    with tile.TileContext(nc) as tc:
        with (
            tc.tile_pool(name="const", bufs=1) as cp,
            tc.tile_pool(name="xt", bufs=2) as xtp,
            tc.tile_pool(name="tbl", bufs=3) as tblp,
            tc.tile_pool(name="gw", bufs=gw_bufs) as gwp,
            tc.tile_pool(name="idx", bufs=idx_bufs) as idxp,
            tc.tile_pool(name="smat", bufs=sp_bufs) as sp,
            tc.tile_pool(name="zz", bufs=2) as zzp,
            tc.tile_pool(name="zw", bufs=zw_bufs) as zwp,
            tc.tile_pool(name="xb", bufs=2) as xbp,
            tc.tile_pool(name="xtb", bufs=2) as xtbp,
            tc.tile_pool(name="hd", bufs=2) as hdp,
            tc.tile_pool(name="ps_acc", bufs=2, space="PSUM") as ps_acc,
            tc.tile_pool(name="ps_tp", bufs=2, space="PSUM") as ps_tp,
            tc.tile_pool(name="ps_tb", bufs=2, space="PSUM") as ps_tb,
            tc.tile_pool(name="ps_lg", bufs=2, space="PSUM") as ps_lg,
            tc.tile_pool(name="dram", bufs=1, space="DRAM") as dp,
        ):
            # ---- constants ----
            M1sb = cp.tile([P, DC, RC], BF16)
            M2sb = cp.tile([P, HC, RC], BF16)
            WcSb = cp.tile([P, HC, OUT], BF16)
            b1sb = cp.tile([P, HID], F32)
            b2sb = cp.tile([P, HID], F32)
            bcsb = cp.tile([P, OUT], F32)
            iota = cp.tile([P, P], BF16)
            ident = cp.tile([P, P], BF16)
            adt1 = cp.tile([P, NB * H], F32)
            adt2 = cp.tile([P, NB * H], F32)
            nc.sync.dma_start(out=M1sb[:], in_=t_M1[:, :].rearrange(
                "(a c) r -> c a r", c=P))
            nc.sync.dma_start(out=M2sb[:], in_=t_M2[:, :].rearrange(
                "(a c) r -> c a r", c=P))
            nc.sync.dma_start(out=WcSb[:], in_=t_Wc[:, :].rearrange(
                "(a c) r -> c a r", c=P))
            nc.sync.dma_start(out=b1sb[:], in_=t_b1[:, :])
            nc.sync.dma_start(out=b2sb[:], in_=t_b2[:, :])
            nc.sync.dma_start(out=bcsb[:], in_=t_bc[:, :])
            nc.sync.dma_start(out=iota[:], in_=t_iota[:, :])
            make_identity(nc, ident[:])
            nc.gpsimd.memset(adt1[:], 0.0)
            nc.gpsimd.memset(adt2[:], 0.0)

            # ---- internal DRAM ----
            ag1_in = dp.tile([npc, RT], BF16)
            ag2_in = dp.tile([npc, RT], BF16)
            if shared_tables:
                table1 = nc.dram_tensor("table1s", [N, RT], BF16,
                                        kind="Internal", addr_space="Shared")
                table2 = nc.dram_tensor("table2s", [N, RT], BF16,
                                        kind="Internal", addr_space="Shared")
            else:
                table1 = dp.tile([N, RT], BF16)
                table2 = dp.tile([N, RT], BF16)

            iota_bc = iota[:].rearrange("p (o i) -> p o i", o=1).to_broadcast(
                [P, UT, P])

            def write_table(g, pstb, ag_in, adt_sb):
                ng = min(P, npc - g * P)
                tbs = tblp.tile([P, RT], BF16, name="tbs")
                nc.vector.tensor_copy(out=tbs[0:ng, 0:HID],
                                      in_=pstb[0:ng, 0:HID])
                # asrc stays f32: bf16 cols 256:272 == f32 cols 128:136
                nc.vector.tensor_copy(
                    out=tbs[0:ng, HID:HID + 2 * H].bitcast(F32),
                    in_=pstb[0:ng, HID:HID + H])
                nc.vector.tensor_copy(out=adt_sb[0:ng, g * H:(g + 1) * H],
                                      in_=pstb[0:ng, HID + H:HID + 2 * H])
                nc.sync.dma_start(out=ag_in[g * P:g * P + ng, :],
                                  in_=tbs[0:ng, :])

            def table1_phase():
                for b in range(NB):
                    ncols = min(P, npc - b * P)
                    xt = xtp.tile([P, DC, P], BF16, name="xt")
                    nc.sync.dma_start(
                        out=xt[:, :, 0:ncols],
                        in_=t_xT[:, b * P:b * P + ncols].rearrange(
                            "(a c) n -> c a n", c=P))
                    pstb = ps_tb.tile([P, RC], F32, name="pstb", tag="pstb")
                    for a in range(DC):
                        nc.tensor.matmul(pstb[0:ncols, :], xt[:, a, 0:ncols],
                                         M1sb[:, a, :], start=(a == 0),
                                         stop=(a == DC - 1))
                    write_table(b, pstb, ag1_in, adt1)

            def allgather(ag_in, table):
                if no_collectives:
                    nc.sync.dma_start(out=table[0:npc, :], in_=ag_in[:])
                else:
                    nc.gpsimd.collective_compute(
                        "AllGather", mybir.AluOpType.bypass,
                        replica_groups=rgroups,
                        ins=[ag_in[:].opt()], outs=[table[:].opt()])

            # ---- edge phase ----
            def edge_phase(table_full, adt_sb, flush_fn):
                qn = [0]

                def front(g):
                    ix = idxp.tile([P, UT * 8], I16, name="ix")
                    nc.sync.dma_start(out=ix[:], in_=t_idx[g])
                    dse = idxp.tile([P, 3 * UT], BF16, name="dse")
                    nc.sync.dma_start(out=dse[:], in_=t_dse[g])

                    def chunked_gather(dst_tile, src_ap, idx_ap, u, off):
                        for c0 in range(0, u, ch):
                            c1 = min(c0 + ch, u)
                            n = (c1 - c0) * P
                            qn[0] = (qn[0] + 1) % sq
                            if "densegather" in pert:
                                nc.sync.dma_start(
                                    out=dst_tile[:, off + c0:off + c1, :],
                                    in_=src_ap.tensor.ap()[0:n, 0:RT].rearrange(
                                        "(k p) r -> p k r", p=P))
                            else:
                                nc.gpsimd.dma_gather(
                                    dst_tile[:, off + c0:off + c1, :], src_ap,
                                    idx_ap[:, (off + c0) * 8:(off + c1) * 8],
                                    n, n, RT, queue_num=qn[0])

                    gw = gwp.tile([P, UT, RT], BF16, name="gw")
                    chunked_gather(gw, table_full[0:NA_ROWS, :], ix, UA, 0)
                    if cfg.split:
                        chunked_gather(gw, table_full[SPLIT_AT:N, :], ix, UB,
                                       UA)

                    # St [e-part, d-col]; StT [d-part, e-col]
                    St = sp.tile([P, UT, P], BF16, name="St")
                    if sel_ship >= 2:
                        nc.sync.dma_start(
                            out=St[:], in_=t_stm[g].rearrange(
                                "p (j d) -> p j d", d=P))
                    else:
                        dc_bc = dse[:, 0:UT].rearrange(
                            "p (j o) -> p j o", o=1).to_broadcast([P, UT, P])
                        nc.vector.tensor_tensor(out=St[:], in0=dc_bc,
                                                in1=iota_bc,
                                                op=mybir.AluOpType.is_equal)
                    StT = sp.tile([P, UT, P], BF16, name="StT")
                    if sel_ship >= 1:
                        (nc.scalar if stt_q else nc.sync).dma_start(
                            out=StT[:], in_=t_stt[g].rearrange(
                                "p (j e) -> p j e", e=P))
                    else:
                        st_bc = dse[:, UT:2 * UT].rearrange(
                            "p (j o) -> p j o", o=1).to_broadcast([P, UT, P])
                        en_bc = dse[:, 2 * UT:3 * UT].rearrange(
                            "p (j o) -> p j o", o=1).to_broadcast([P, UT, P])
                        g1 = sp.tile([P, UT, P], BF16, name="g1")
                        nc.vector.tensor_tensor(out=g1[:], in0=iota_bc,
                                                in1=st_bc,
                                                op=mybir.AluOpType.is_ge)
                        nc.vector.tensor_tensor(out=StT[:], in0=iota_bc,
                                                in1=en_bc,
                                                op=mybir.AluOpType.is_lt)
                        nc.vector.tensor_tensor(out=StT[:], in0=StT[:],
                                                in1=g1[:],
                                                op=mybir.AluOpType.mult)

                    adst_bf = zzp.tile([P, H], BF16, name="adst_bf")
                    nc.vector.tensor_copy(out=adst_bf[:],
                                          in_=adt_sb[:, g * H:(g + 1) * H])
                    # acc bank tile: [0:264] segment acc | [264:424] adstE
                    accb = ps_acc.tile([P, HID + H + UT * H], F32, name="accb")
                    psad = accb[:, HID + H:HID + H + UT * H].rearrange(
                        "p (j h) -> p j h", h=H)
                    for j in range(UT):
                        nc.tensor.matmul(psad[:, j, :], StT[:, j, :],
                                         adst_bf[:], start=True, stop=True)

                    # zt = lrelu(asrc + adstE) -> e = exp(zt)
                    zt = zzp.tile([P, UT, H], F32, name="zt")
                    nc.vector.tensor_add(
                        out=zt[:],
                        in0=gw[:, :, HID:HID + 2 * H].bitcast(F32),
                        in1=psad[:])
                    if act_lrelu:
                        nc.scalar.activation(zt[:], zt[:],
                                             mybir.ActivationFunctionType.Lrelu,
                                             alpha=0.2)
                    else:
                        nc.vector.scalar_tensor_tensor(
                            out=zt[:], in0=zt[:], scalar=0.2, in1=zt[:],
                            op0=mybir.AluOpType.mult, op1=mybir.AluOpType.max)
                    zw = zwp.tile([P, UT, HID + H], BF16, name="zw")
                    nc.scalar.activation(zw[:, :, HID:HID + H], zt[:],
                                         mybir.ActivationFunctionType.Exp)
                    e_bc = zw[:, :, HID:HID + H].to_broadcast([P, UT, H, C])
                    hv = zw[:, :, 0:HID].rearrange("p k (h c) -> p k h c", c=C)
                    nc.vector.tensor_tensor(
                        out=hv,
                        in0=gw[:, :, 0:HID].rearrange("p k (h c) -> p k h c",
                                                      c=C),
                        in1=e_bc, op=mybir.AluOpType.mult)

                    acc = accb[:, 0:HID + H]
                    js = [0] if "mm1" in pert else list(range(UT))
                    if dual_acc:
                        acc2t = ps_lg.tile([P, HID + H], F32, name="acc2")
                        js0 = js[0::2]
                        js1 = js[1::2]
                        for j in js0:
                            nc.tensor.matmul(acc[:], St[:, j, :], zw[:, j, :],
                                             start=(j == js0[0]),
                                             stop=(j == js0[-1]))
                        for j in js1:
                            nc.tensor.matmul(acc2t[:], St[:, j, :],
                                             zw[:, j, :], start=(j == js1[0]),
                                             stop=(j == js1[-1]))
                        accs = xbp.tile([P, HID + H], F32, name="accs")
                        nc.vector.tensor_add(out=accs[:], in0=acc[:],
                                             in1=acc2t[:])
                        acc_out = accs
                    else:
                        for j in js:
                            nc.tensor.matmul(acc[:], St[:, j, :], zw[:, j, :],
                                             start=(j == js[0]),
                                             stop=(j == js[-1]) or "mm1" in pert)
                        acc_out = accb
                    return acc_out

                pend = []
                for g in range(NB):
                    pend.append((g, front(g)))
                    if len(pend) > skew:
                        gq, accq = pend.pop(0)
                        if "noflush" not in pert:
                            flush_fn(gq, accq)
                for gq, accq in pend:
                    if "noflush" not in pert:
                        flush_fn(gq, accq)

            # ---- flush helpers ----
            def normalize(acc, bias_sb):
                rec = zzp.tile([P, H], F32, name="rec")
                nc.vector.reciprocal(rec[:], acc[:, HID:HID + H])
                xb = xbp.tile([P, HID], F32, name="xb")
                nc.vector.tensor_mul(
                    out=xb[:].rearrange("p (h c) -> p h c", c=C),
                    in0=acc[:, 0:HID].rearrange("p (h c) -> p h c", c=C),
                    in1=rec[:].to_broadcast([P, H, C]))
                nc.vector.tensor_add(out=xb[:], in0=xb[:], in1=bias_sb[:])
                xbb = xbp.tile([P, HID], BF16, name="xbb")
                nc.vector.tensor_scalar_max(xbb[:], xb[:], 0.0)
                return xbb

            def transpose2(xbb):
                outs = []
                for a in range(HC):
                    pst = ps_tp.tile([P, P], BF16, name="pst")
                    nc.tensor.transpose(pst[:], xbb[:, a * P:(a + 1) * P],
                                        ident[:])
                    xts = xtbp.tile([P, P], BF16, name="xts")
                    nc.vector.tensor_copy(out=xts[:], in_=pst[:])
                    outs.append(xts)
                return outs

            def flush_layer1(g, acc):
                ng = min(P, npc - g * P)
                xbb = normalize(acc, b1sb)
                xts = transpose2(xbb)
                pstb = ps_tb.tile([P, RC], F32, name="pstb2", tag="pstb")
                for a in range(HC):
                    nc.tensor.matmul(pstb[0:ng, :], xts[a][:, 0:ng],
                                     M2sb[:, a, :], start=(a == 0),
                                     stop=(a == HC - 1))
                write_table(g, pstb, ag2_in, adt2)

            def flush_layer2(g, acc):
                ng = min(P, npc - g * P)
                xbb = normalize(acc, b2sb)
                xts = transpose2(xbb)
                pslg = ps_lg.tile([P, OUT], F32, name="pslg")
                for a in range(HC):
                    nc.tensor.matmul(pslg[0:ng, :], xts[a][:, 0:ng],
                                     WcSb[:, a, :], start=(a == 0),
                                     stop=(a == HC - 1))
                lg = hdp.tile([P, OUT], F32, name="lg")
                nc.vector.tensor_add(out=lg[0:ng, :], in0=pslg[0:ng, :],
                                     in1=bcsb[0:ng, :])
                mx = hdp.tile([P, 1], F32, name="mx")
                nc.vector.tensor_reduce(out=mx[0:ng, :], in_=lg[0:ng, :],
                                        axis=mybir.AxisListType.X,
                                        op=mybir.AluOpType.max)
                nc.vector.tensor_sub(out=lg[0:ng, :], in0=lg[0:ng, :],
                                     in1=mx[0:ng, :].to_broadcast([ng, OUT]))
                ex = hdp.tile([P, OUT], F32, name="ex")
                dn = hdp.tile([P, 1], F32, name="dn")
                nc.scalar.activation(ex[0:ng, :], lg[0:ng, :],
                                     mybir.ActivationFunctionType.Exp,
                                     accum_out=dn[0:ng, :])
                lnd = hdp.tile([P, 1], F32, name="lnd")
                nc.scalar.activation(lnd[0:ng, :], dn[0:ng, :],
                                     mybir.ActivationFunctionType.Ln)
                ob = hdp.tile([P, OUT], F32, name="ob")
                nc.vector.tensor_sub(out=ob[0:ng, :], in0=lg[0:ng, :],
                                     in1=lnd[0:ng, :].to_broadcast([ng, OUT]))
                nc.sync.dma_start(out=t_out[g * P:g * P + ng, :],
                                  in_=ob[0:ng, :])

            for _ in range(kernel_reps):
                table1_phase()
                allgather(ag1_in, table1)
                for _ in range(edge_reps):
                    edge_phase(table1, adt1, flush_layer1)
                allgather(ag2_in, table2)
                for _ in range(edge_reps):
                    edge_phase(table2, adt2, flush_layer2)

    nc.compile()
    return nc


def _bf16(a):
    import ml_dtypes
    return np.asarray(a, dtype=ml_dtypes.bfloat16)


def make_in_maps(cfg: Cfg, pre, x, W1, as1, ad1, b1, W2, as2, ad2, b2, Wc, bc):
    H, C, HID, npc, RC = cfg.H, cfg.C, cfg.HID, cfg.npc, cfg.RC

    def mk_m(W, a_s, a_d):
        M = np.zeros((W.shape[0], RC), np.float32)
        M[:, 0:HID] = W
        M[:, HID:HID + H] = W @ expand_att(a_s, HID, H, C)
        M[:, HID + H:HID + 2 * H] = W @ expand_att(a_d, HID, H, C)
        return _bf16(M)

    M1 = mk_m(W1, as1, ad1)
    M2 = mk_m(W2, as2, ad2)
    iota = _bf16(np.tile(np.arange(P, dtype=np.float32)[None, :], (P, 1)))
    maps = []
    for c in range(cfg.n_cores):
        maps.append({
            "xT": _bf16(np.ascontiguousarray(x[c * npc:(c + 1) * npc].T)),
            "M1": M1, "M2": M2, "Wc": _bf16(Wc),
            "b1": np.tile(b1[None, :], (P, 1)).astype(np.float32),
            "b2": np.tile(b2[None, :], (P, 1)).astype(np.float32),
            "bc": np.tile(bc[None, :], (P, 1)).astype(np.float32),
            "iota": iota,
            "idx": pre[c]["idx"],
            "dse": _bf16(pre[c]["dse"]),
            "stt": _bf16(pre[c]["stt"].reshape(cfg.NB, P, -1)),
            "stm": _bf16(pre[c]["stm"].reshape(cfg.NB, P, -1)),
        })
    return maps


def kernel(x, edge_index, W1, as1, ad1, b1, W2, as2, ad2, b2, Wc, bc):
    x = np.asarray(x, dtype=np.float32)
    edge_index = np.asarray(edge_index)
    N, DIN = x.shape
    H, C = np.asarray(as1).shape
    OUT = np.asarray(Wc).shape[1]
    n_cores = 8

    cfg = Cfg(N, DIN, H, C, OUT, n_cores)
    pre = preprocess(cfg, edge_index)
    nc = build_program(cfg, sel_ship=1, shared_tables=1)
    in_maps = make_in_maps(cfg, pre, x,
                           np.asarray(W1, np.float32), np.asarray(as1, np.float32),
                           np.asarray(ad1, np.float32), np.asarray(b1, np.float32),
                           np.asarray(W2, np.float32), np.asarray(as2, np.float32),
                           np.asarray(ad2, np.float32), np.asarray(b2, np.float32),
                           np.asarray(Wc, np.float32), np.asarray(bc, np.float32))

    from concourse import bass_utils
    last_err = None
    for _attempt in range(3):
        try:
            res = bass_utils.run_bass_kernel_spmd(nc, in_maps,
                                                  core_ids=list(range(n_cores)))
            break
        except Exception as e:                      # noqa: BLE001
            last_err = e
    else:
        raise last_err
    return np.concatenate([res.results[c]["out"] for c in range(n_cores)],
                          axis=0)
